# revision 2
# baseline (speedup 1.0000x reference)
"""Trainium2 Bass kernel for the Clifford EP model.

The reference model is entirely linear in x_mv:
  * Wx = geometric_product(x, W_in) is linear (Cayley-table contraction).
  * The free-phase relaxation h <- h + dt*(Wx - h), h0 = 0, has the exact
    closed form h_free = (1 - (1-dt)^N) * Wx.
  * The output is the scalar blade of geometric_product(h_free, W_out),
    and C[a, c, 0] != 0 only for c == a.

So the whole network collapses to a single matmul
    out[b, o] = X[b, :] @ Mf[:, o]
with X = x_mv.reshape(B, M*I) and a (M*I, O) folded weight matrix Mf that
only depends on W_in, W_out and the Cayley table.  The fold itself is tiny
(512x4096 @ 4096x64) and is done once on the host in float64; the device
does the batch-sized work: a data-parallel (1024x512)@(512x64) matmul per
NeuronCore, which is purely input-bandwidth bound (2 MB of X per core).

Device layout: each core receives X_shard transposed (k on partitions) so
the TensorEngine can contract over k directly:
    psum[o, b] += Mf_chunk[k,o].T @ XT_chunk[k, b]
accumulated over 4 k-chunks of 128, with the 1024-batch free dim split in
two 512-wide matmuls (fp32 moving-operand limit / one PSUM bank).
"""

import numpy as np

# Model constants (hardcoded per the problem spec).
B, M_DIM, I_B = 8192, 64, 8
H_DIM, O_DIM = 512, 64
K_DIM = M_DIM * I_B  # 512 contraction size
N_CORES = 8
B_SHARD = B // N_CORES  # 1024
DT, N_FREE = 0.1, 20
G_SIG = [1, 1, 1]

_CACHE = {}


def _cayley():
    n = len(G_SIG)
    I = 2**n
    C = np.zeros((I, I, I), dtype=np.float64)
    for a in range(I):
        for b in range(I):
            s = 0
            for i in range(n):
                if (b >> i) & 1:
                    s += bin(a >> (i + 1)).count("1")
            sign = (-1.0) ** s
            common = a & b
            for i in range(n):
                if (common >> i) & 1:
                    sign *= G_SIG[i]
            C[a, b, a ^ b] = sign
    return C


def _fold_weights(W_in, W_out):
    """Collapse W_in, W_out, Cayley table and the relaxation scale into
    a single (K_DIM, O_DIM) float32 matrix Mf with out = X @ Mf."""
    C = _cayley()
    I = I_B
    s = np.array([C[a, a, 0] for a in range(I)])  # scalar-blade signs
    coef = np.zeros((I, I))
    idx = np.zeros((I, I), dtype=np.int64)
    for a in range(I):
        for k in range(I):
            coef[a, k] = C[a, a ^ k, k]
            idx[a, k] = a ^ k
    W_in64 = np.asarray(W_in, dtype=np.float64)
    W_out64 = np.asarray(W_out, dtype=np.float64)
    # U[h, m, a, k] = C[a, a^k, k] * W_in[h, m, a^k]
    U = coef[None, None, :, :] * W_in64[:, :, idx]
    # W2[h, k, o] = s_k * W_out[o, h, k]
    W2 = s[None, :, None] * np.transpose(W_out64, (1, 2, 0))
    Uf = np.transpose(U, (1, 2, 0, 3)).reshape(M_DIM * I, H_DIM * I)
    c0 = 1.0 - (1.0 - DT) ** N_FREE
    Mf = c0 * (Uf @ W2.reshape(H_DIM * I, O_DIM))
    return np.ascontiguousarray(Mf, dtype=np.float32)


def _build_bass():
    """Build + schedule the single-core SPMD program (cached)."""
    if "nc" in _CACHE:
        return _CACHE["nc"]

    import concourse.bacc as bacc
    import concourse.mybir as mybir
    import concourse.tile as tile

    f32 = mybir.dt.float32
    KC = K_DIM // 128  # 4 contraction chunks
    BH = B_SHARD // 512  # 2 free-dim halves

    nc = bacc.Bacc("TRN2", target_bir_lowering=False, debug=False)
    xt = nc.dram_tensor("xt", [K_DIM, B_SHARD], f32, kind="ExternalInput")
    mf = nc.dram_tensor("mf", [K_DIM, O_DIM], f32, kind="ExternalInput")
    out_t = nc.dram_tensor("out_t", [O_DIM, B_SHARD], f32, kind="ExternalOutput")

    with tile.TileContext(nc) as tc:
        with (
            tc.tile_pool(name="consts", bufs=1) as cpool,
            tc.tile_pool(name="out", bufs=2) as opool,
            tc.tile_pool(name="psum", bufs=2, space="PSUM") as ppool,
        ):
            mft = []
            xtt = []
            for kc in range(KC):
                m_t = cpool.tile([128, O_DIM], f32, tag=f"mf{kc}")
                nc.sync.dma_start(out=m_t[:], in_=mf[kc * 128 : (kc + 1) * 128, :])
                mft.append(m_t)
            for kc in range(KC):
                x_t = cpool.tile([128, B_SHARD], f32, tag=f"xt{kc}")
                nc.sync.dma_start(out=x_t[:], in_=xt[kc * 128 : (kc + 1) * 128, :])
                xtt.append(x_t)
            for bh in range(BH):
                ps = ppool.tile([O_DIM, 512], f32, tag="ps")
                for kc in range(KC):
                    nc.tensor.matmul(
                        ps[:],
                        mft[kc][:],
                        xtt[kc][:, bh * 512 : (bh + 1) * 512],
                        start=(kc == 0),
                        stop=(kc == KC - 1),
                    )
                o_t = opool.tile([O_DIM, 512], f32, tag="o")
                nc.vector.tensor_copy(o_t[:], ps[:])
                nc.sync.dma_start(
                    out=out_t[:, bh * 512 : (bh + 1) * 512], in_=o_t[:]
                )

    nc.compile()
    _CACHE["nc"] = nc
    return nc


def _install_ntff_hook_shim():
    """This image's `antenv` lacks `axon_hooks`, which bass_utils imports
    when trace=True under axon.  Recreate it, wired to the ctypes NTFF
    profiler that trn_agent_boot ships.  No-op when the real module exists."""
    import sys
    import types

    try:
        import antenv.axon_hooks  # noqa: F401

        return
    except ImportError:
        pass
    try:
        import antenv
        from trn_agent_boot.trn_boot import _ntff_profile_via_ctypes

        hook = _ntff_profile_via_ctypes("/opt/axon/libaxon_pjrt.so")
    except Exception:
        antenv, hook = None, None
    if antenv is None:
        return
    mod = types.ModuleType("antenv.axon_hooks")
    mod.get_axon_ntff_profile_hook = lambda: hook
    mod.set_axon_ntff_profile_hook = lambda h: None
    sys.modules["antenv.axon_hooks"] = mod
    antenv.axon_hooks = mod


def kernel(x_mv, W_in, W_out, trace=False, **trace_kwargs):
    _install_ntff_hook_shim()
    from concourse.bass_utils import run_bass_kernel_spmd

    x_mv = np.asarray(x_mv, dtype=np.float32)
    Mf = _fold_weights(W_in, W_out)

    X = x_mv.reshape(B, K_DIM)
    in_maps = []
    for c in range(N_CORES):
        xs = np.ascontiguousarray(X[c * B_SHARD : (c + 1) * B_SHARD].T)
        in_maps.append({"xt": xs, "mf": Mf})

    nc = _build_bass()
    res = run_bass_kernel_spmd(
        nc, in_maps, core_ids=list(range(N_CORES)), trace=trace, **trace_kwargs
    )
    _CACHE["last_results"] = res

    out = np.empty((B, O_DIM), dtype=np.float32)
    for c in range(N_CORES):
        out[c * B_SHARD : (c + 1) * B_SHARD] = res.results[c]["out_t"].T
    return out


# revision 4
# speedup vs baseline: 1.0217x; 1.0217x over previous
"""Trainium2 Bass kernel for the Clifford EP model.

The reference model is entirely linear in x_mv:
  * Wx = geometric_product(x, W_in) is linear (Cayley-table contraction).
  * The free-phase relaxation h <- h + dt*(Wx - h), h0 = 0, has the exact
    closed form h_free = (1 - (1-dt)^N) * Wx.
  * The output is the scalar blade of geometric_product(h_free, W_out),
    and C[a, c, 0] != 0 only for c == a.

So the whole network collapses to a single matmul
    out[b, o] = X[b, :] @ Mf[:, o]
with X = x_mv.reshape(B, M*I) and a (M*I, O) folded weight matrix Mf that
only depends on W_in, W_out and the Cayley table.  The fold itself is tiny
(512x4096 @ 4096x64) and is done once on the host in float64; the device
does the batch-sized work: a data-parallel (1024x512)@(512x64) matmul per
NeuronCore, which is purely input-bandwidth bound (2 MB of X per core).

Device layout: each core receives X_shard transposed (k on partitions) so
the TensorEngine can contract over k directly:
    psum[o, b] += Mf_chunk[k,o].T @ XT_chunk[k, b]
accumulated over 4 k-chunks of 128, with the 1024-batch free dim split in
two 512-wide matmuls (fp32 moving-operand limit / one PSUM bank).
"""

import numpy as np

# Model constants (hardcoded per the problem spec).
B, M_DIM, I_B = 8192, 64, 8
H_DIM, O_DIM = 512, 64
K_DIM = M_DIM * I_B  # 512 contraction size
N_CORES = 8
B_SHARD = B // N_CORES  # 1024
DT, N_FREE = 0.1, 20
G_SIG = [1, 1, 1]

_CACHE = {}


def _cayley():
    n = len(G_SIG)
    I = 2**n
    C = np.zeros((I, I, I), dtype=np.float64)
    for a in range(I):
        for b in range(I):
            s = 0
            for i in range(n):
                if (b >> i) & 1:
                    s += bin(a >> (i + 1)).count("1")
            sign = (-1.0) ** s
            common = a & b
            for i in range(n):
                if (common >> i) & 1:
                    sign *= G_SIG[i]
            C[a, b, a ^ b] = sign
    return C


def _fold_weights(W_in, W_out):
    """Collapse W_in, W_out, Cayley table and the relaxation scale into
    a single (K_DIM, O_DIM) float32 matrix Mf with out = X @ Mf."""
    C = _cayley()
    I = I_B
    s = np.array([C[a, a, 0] for a in range(I)])  # scalar-blade signs
    coef = np.zeros((I, I))
    idx = np.zeros((I, I), dtype=np.int64)
    for a in range(I):
        for k in range(I):
            coef[a, k] = C[a, a ^ k, k]
            idx[a, k] = a ^ k
    W_in64 = np.asarray(W_in, dtype=np.float64)
    W_out64 = np.asarray(W_out, dtype=np.float64)
    # U[h, m, a, k] = C[a, a^k, k] * W_in[h, m, a^k]
    U = coef[None, None, :, :] * W_in64[:, :, idx]
    # W2[h, k, o] = s_k * W_out[o, h, k]
    W2 = s[None, :, None] * np.transpose(W_out64, (1, 2, 0))
    Uf = np.transpose(U, (1, 2, 0, 3)).reshape(M_DIM * I, H_DIM * I)
    c0 = 1.0 - (1.0 - DT) ** N_FREE
    Mf = c0 * (Uf @ W2.reshape(H_DIM * I, O_DIM))
    return np.ascontiguousarray(Mf, dtype=np.float32)


def _build_bass():
    """Build + schedule the single-core SPMD program (cached)."""
    if "nc" in _CACHE:
        return _CACHE["nc"]

    import concourse.bacc as bacc
    import concourse.mybir as mybir
    import concourse.tile as tile

    f32 = mybir.dt.float32
    KC = K_DIM // 128  # 4 contraction chunks
    BH = B_SHARD // 512  # 2 free-dim halves

    nc = bacc.Bacc("TRN2", target_bir_lowering=False, debug=False)
    xt = nc.dram_tensor("xt", [K_DIM, B_SHARD], f32, kind="ExternalInput")
    mf = nc.dram_tensor("mf", [K_DIM, O_DIM], f32, kind="ExternalInput")
    out_t = nc.dram_tensor("out_t", [O_DIM, B_SHARD], f32, kind="ExternalOutput")

    bf16 = mybir.dt.bfloat16
    N_WARM = 26  # dummy matmuls to flip the PE HAM clock-gate to 2.4 GHz

    with tile.TileContext(nc) as tc:
        with (
            tc.tile_pool(name="consts", bufs=1) as cpool,
            tc.tile_pool(name="out", bufs=2) as opool,
            tc.tile_pool(name="psum", bufs=2, space="PSUM") as ppool,
            tc.tile_pool(name="warm", bufs=1) as wpool,
            tc.tile_pool(name="warmps", bufs=1, space="PSUM") as wppool,
        ):
            # PE warmup: HAM un-throttles only after ~3.4us of sustained PE
            # activity.  Dummy bf16 matmuls (no data deps) run while the xt
            # DMAs stream, so the real fp32 matmuls execute at 2.4 GHz.
            w_in = wpool.tile([128, 512], bf16, tag="warm_in")
            w_ps = wppool.tile([128, 512], f32, tag="warm_ps")
            nc.gpsimd.memset(w_in[:], 0.0)
            for _ in range(N_WARM):
                nc.tensor.matmul(w_ps[:], w_in[:, :128], w_in[:], start=True, stop=True)

            # One merged mf load ([512,64] -> [128, 4x64]) + per-chunk xt
            # loads so the PE can start accumulating as chunks arrive.
            mft = cpool.tile([128, KC, O_DIM], f32, tag="mf")
            nc.sync.dma_start(
                out=mft[:], in_=mf.rearrange("(c p) o -> p c o", p=128)
            )
            xtt = []
            for kc in range(KC):
                x_t = cpool.tile([128, B_SHARD], f32, tag=f"xt{kc}")
                nc.sync.dma_start(out=x_t[:], in_=xt[kc * 128 : (kc + 1) * 128, :])
                xtt.append(x_t)

            # kc-outer: each chunk feeds both PSUM accumulation groups as
            # soon as its DMA lands.
            pss = [
                ppool.tile([O_DIM, 512], f32, name=f"ps{bh}", tag=f"ps{bh}")
                for bh in range(BH)
            ]
            for kc in range(KC):
                for bh in range(BH):
                    nc.tensor.matmul(
                        pss[bh][:],
                        mft[:, kc, :],
                        xtt[kc][:, bh * 512 : (bh + 1) * 512],
                        start=(kc == 0),
                        stop=(kc == KC - 1),
                    )
            for bh in range(BH):
                o_t = opool.tile([O_DIM, 512], f32, tag="o")
                nc.vector.tensor_copy(o_t[:], pss[bh][:])
                nc.sync.dma_start(
                    out=out_t[:, bh * 512 : (bh + 1) * 512], in_=o_t[:]
                )

    nc.compile()
    _CACHE["nc"] = nc
    return nc


def _install_ntff_hook_shim():
    """This image's `antenv` lacks `axon_hooks`, which bass_utils imports
    when trace=True under axon.  Recreate it, wired to the ctypes NTFF
    profiler that trn_agent_boot ships.  No-op when the real module exists."""
    import sys
    import types

    try:
        import antenv.axon_hooks  # noqa: F401

        return
    except ImportError:
        pass
    try:
        import antenv
        from trn_agent_boot.trn_boot import _ntff_profile_via_ctypes

        hook = _ntff_profile_via_ctypes("/opt/axon/libaxon_pjrt.so")
    except Exception:
        antenv, hook = None, None
    if antenv is None:
        return
    mod = types.ModuleType("antenv.axon_hooks")
    mod.get_axon_ntff_profile_hook = lambda: hook
    mod.set_axon_ntff_profile_hook = lambda h: None
    sys.modules["antenv.axon_hooks"] = mod
    antenv.axon_hooks = mod


def kernel(x_mv, W_in, W_out, trace=False, **trace_kwargs):
    _install_ntff_hook_shim()
    from concourse.bass_utils import run_bass_kernel_spmd

    x_mv = np.asarray(x_mv, dtype=np.float32)
    Mf = _fold_weights(W_in, W_out)

    X = x_mv.reshape(B, K_DIM)
    in_maps = []
    for c in range(N_CORES):
        xs = np.ascontiguousarray(X[c * B_SHARD : (c + 1) * B_SHARD].T)
        in_maps.append({"xt": xs, "mf": Mf})

    nc = _build_bass()
    res = run_bass_kernel_spmd(
        nc, in_maps, core_ids=list(range(N_CORES)), trace=trace, **trace_kwargs
    )
    _CACHE["last_results"] = res

    out = np.empty((B, O_DIM), dtype=np.float32)
    for c in range(N_CORES):
        out[c * B_SHARD : (c + 1) * B_SHARD] = res.results[c]["out_t"].T
    return out


# revision 5
# speedup vs baseline: 1.4303x; 1.3999x over previous
"""Trainium2 Bass kernel for the Clifford EP model.

The reference model is entirely linear in x_mv:
  * Wx = geometric_product(x, W_in) is linear (Cayley-table contraction).
  * The free-phase relaxation h <- h + dt*(Wx - h), h0 = 0, has the exact
    closed form h_free = (1 - (1-dt)^N) * Wx.
  * The output is the scalar blade of geometric_product(h_free, W_out),
    and C[a, c, 0] != 0 only for c == a.

So the whole network collapses to a single matmul
    out[b, o] = X[b, :] @ Mf[:, o]
with X = x_mv.reshape(B, M*I) and a (M*I, O) folded weight matrix Mf that
only depends on W_in, W_out and the Cayley table.  The fold itself is tiny
(512x4096 @ 4096x64) and is done once on the host in float64; the device
does the batch-sized work: a data-parallel (1024x512)@(512x64) matmul per
NeuronCore, which is purely input-bandwidth bound.

Device layout: each core receives X_shard transposed (k on partitions) so
the TensorEngine can contract over k directly:
    psum[o, b] += Mf_chunk[k,o].T @ XT_chunk[k, b]
accumulated over 4 k-chunks of 128, with the 1024-batch free dim split in
two 512-wide matmuls (one PSUM bank each).

The device data path is fp16 (PSUM accumulation stays fp32): one PE pass
per matmul (fp32 needs LOW/HIGH double passes at half stream rate) and
half the DMA bytes.  Measured end-to-end relative error ~3e-4.
Set dtype="f32" in kernel() for the exact fp32 path.

Raw Bass (no TileContext) with manual semaphores: the Tile scheduler's
drain + double all-engine barrier + semaphore-clear tail costs ~7us,
which is material at this kernel size.
"""

import numpy as np

# Model constants (hardcoded per the problem spec).
B, M_DIM, I_B = 8192, 64, 8
H_DIM, O_DIM = 512, 64
K_DIM = M_DIM * I_B  # 512 contraction size
N_CORES = 8
B_SHARD = B // N_CORES  # 1024
KC = K_DIM // 128  # 4 contraction chunks
BH = B_SHARD // 512  # 2 moving-operand halves
DT, N_FREE = 0.1, 20
G_SIG = [1, 1, 1]

_CACHE = {}


def _cayley():
    n = len(G_SIG)
    I = 2**n
    C = np.zeros((I, I, I), dtype=np.float64)
    for a in range(I):
        for b in range(I):
            s = 0
            for i in range(n):
                if (b >> i) & 1:
                    s += bin(a >> (i + 1)).count("1")
            sign = (-1.0) ** s
            common = a & b
            for i in range(n):
                if (common >> i) & 1:
                    sign *= G_SIG[i]
            C[a, b, a ^ b] = sign
    return C


def _fold_weights(W_in, W_out):
    """Collapse W_in, W_out, Cayley table and the relaxation scale into
    a single (K_DIM, O_DIM) float64 matrix Mf with out = X @ Mf."""
    C = _cayley()
    I = I_B
    s = np.array([C[a, a, 0] for a in range(I)])  # scalar-blade signs
    coef = np.zeros((I, I))
    idx = np.zeros((I, I), dtype=np.int64)
    for a in range(I):
        for k in range(I):
            coef[a, k] = C[a, a ^ k, k]
            idx[a, k] = a ^ k
    W_in64 = np.asarray(W_in, dtype=np.float64)
    W_out64 = np.asarray(W_out, dtype=np.float64)
    # U[h, m, a, k] = C[a, a^k, k] * W_in[h, m, a^k]
    U = coef[None, None, :, :] * W_in64[:, :, idx]
    # W2[h, k, o] = s_k * W_out[o, h, k]
    W2 = s[None, :, None] * np.transpose(W_out64, (1, 2, 0))
    Uf = np.transpose(U, (1, 2, 0, 3)).reshape(M_DIM * I, H_DIM * I)
    c0 = 1.0 - (1.0 - DT) ** N_FREE
    return c0 * (Uf @ W2.reshape(H_DIM * I, O_DIM))


def _install_ntff_hook_shim():
    """This image's `antenv` lacks `axon_hooks`, which bass_utils imports
    when trace=True under axon.  Recreate it, wired to the ctypes NTFF
    profiler that trn_agent_boot ships.  No-op when the real module exists."""
    import sys
    import types

    try:
        import antenv.axon_hooks  # noqa: F401

        return
    except ImportError:
        pass
    try:
        import antenv
        from trn_agent_boot.trn_boot import _ntff_profile_via_ctypes

        hook = _ntff_profile_via_ctypes("/opt/axon/libaxon_pjrt.so")
    except Exception:
        antenv, hook = None, None
    if antenv is None:
        return
    mod = types.ModuleType("antenv.axon_hooks")
    mod.get_axon_ntff_profile_hook = lambda: hook
    mod.set_axon_ntff_profile_hook = lambda h: None
    sys.modules["antenv.axon_hooks"] = mod
    antenv.axon_hooks = mod


def _build_bass(dtype_key, n_warm):
    """Build the single-core SPMD program with raw-bass manual sync."""
    key = ("nc", dtype_key, n_warm)
    if key in _CACHE:
        return _CACHE[key]

    import concourse.bass as bass
    import concourse.mybir as mybir

    f32 = mybir.dt.float32
    dt_in = {"f16": mybir.dt.float16, "f32": f32, "bf16": mybir.dt.bfloat16}[
        dtype_key
    ]

    nc = bass.Bass("TRN2", debug=False)
    xt = nc.dram_tensor("xt", [K_DIM, B_SHARD], dt_in, kind="ExternalInput")
    mf = nc.dram_tensor("mf", [K_DIM, O_DIM], dt_in, kind="ExternalInput")
    out_t = nc.dram_tensor("out_t", [O_DIM, B_SHARD], f32, kind="ExternalOutput")

    with (
        nc.sbuf_tensor([128, KC, B_SHARD], dt_in) as xts,
        nc.sbuf_tensor([128, KC, O_DIM], dt_in) as mft,
        nc.sbuf_tensor([128, 512], mybir.dt.bfloat16) as warm_w,
        nc.sbuf_tensor([O_DIM, B_SHARD], f32) as o_sb,
        nc.psum_tensor([O_DIM, BH, 512], f32) as ps,
        nc.psum_tensor([128, 512], f32) as warm_ps,
        nc.semaphore("sem_mf") as sem_mf,
        nc.semaphore("sem_warm") as sem_warm,
        nc.semaphore("sem_xt0") as sem_xt0,
        nc.semaphore("sem_xt1") as sem_xt1,
        nc.semaphore("sem_xt2") as sem_xt2,
        nc.semaphore("sem_xt3") as sem_xt3,
        nc.semaphore("sem_mm") as sem_mm,
        nc.semaphore("sem_cp") as sem_cp,
        nc.semaphore("sem_out") as sem_out,
        nc.Block() as block,
    ):
        sem_xt = [sem_xt0, sem_xt1, sem_xt2, sem_xt3]
        mf_view = mf.rearrange("(c p) o -> p c o", p=128)

        @block.gpsimd
        def _(gpsimd):
            gpsimd.memset(warm_w[:], 0.0).then_inc(sem_warm, 1)

        @block.sync
        def _(sync):
            # xt chunk loads; chunk order matches PE consumption order.
            for kc in (0, 1):
                sync.dma_start(
                    out=xts[:, kc, :], in_=xt[kc * 128 : (kc + 1) * 128, :]
                ).then_inc(sem_xt[kc], 16)
            # Output stores as soon as each PSUM half is copied out.
            for bh in range(BH):
                sync.wait_ge(sem_cp, bh + 1)
                sync.dma_start(
                    out=out_t[:, bh * 512 : (bh + 1) * 512], in_=o_sb[:, bh * 512 : (bh + 1) * 512]
                ).then_inc(sem_out, 16)
            sync.wait_ge(sem_out, 32 )

        @block.scalar
        def _(scalar):
            # Second HWDGE issuer, in parallel with sync.
            scalar.dma_start(out=mft[:], in_=mf_view).then_inc(sem_mf, 16)
            for kc in (2, 3):
                scalar.dma_start(
                    out=xts[:, kc, :], in_=xt[kc * 128 : (kc + 1) * 128, :]
                ).then_inc(sem_xt[kc], 16)

        @block.tensor
        def _(tensor):
            # Warm the PE HAM clock-gate while the DMAs stream.
            if n_warm:
                tensor.wait_ge(sem_warm, 1)
                for _ in range(n_warm):
                    nc.tensor.matmul(
                        warm_ps[:], warm_w[:, :128], warm_w[:], start=True, stop=True
                    )
            tensor.wait_ge(sem_mf, 16)
            for kc in range(KC):
                tensor.wait_ge(sem_xt[kc], 16)
                for bh in range(BH):
                    mm = nc.tensor.matmul(
                        ps[:, bh, :],
                        mft[:, kc, :],
                        xts[:, kc, bh * 512 : (bh + 1) * 512],
                        start=(kc == 0),
                        stop=(kc == KC - 1),
                    )
                    if kc == KC - 1:
                        mm.then_inc(sem_mm, 1)

        @block.vector
        def _(vector):
            for bh in range(BH):
                vector.wait_ge(sem_mm, bh + 1)
                nc.vector.tensor_copy(
                    o_sb[:, bh * 512 : (bh + 1) * 512], ps[:, bh, :]
                ).then_inc(sem_cp, 1)

    _CACHE[key] = nc
    return nc


def kernel(x_mv, W_in, W_out, trace=False, dtype="f16", n_warm=4, **trace_kwargs):
    _install_ntff_hook_shim()
    from concourse.bass_utils import run_bass_kernel_spmd

    np_dt = {"f16": np.float16, "f32": np.float32, "bf16": None}[dtype]
    if np_dt is None:
        import ml_dtypes

        np_dt = ml_dtypes.bfloat16

    x_mv = np.asarray(x_mv, dtype=np.float32)
    Mf = np.ascontiguousarray(_fold_weights(W_in, W_out), dtype=np_dt)

    X = x_mv.reshape(B, K_DIM)
    in_maps = []
    for c in range(N_CORES):
        xs = np.ascontiguousarray(X[c * B_SHARD : (c + 1) * B_SHARD].T.astype(np_dt))
        in_maps.append({"xt": xs, "mf": Mf})

    nc = _build_bass(dtype, n_warm)
    res = run_bass_kernel_spmd(
        nc, in_maps, core_ids=list(range(N_CORES)), trace=trace, **trace_kwargs
    )
    _CACHE["last_results"] = res

    out = np.empty((B, O_DIM), dtype=np.float32)
    for c in range(N_CORES):
        out[c * B_SHARD : (c + 1) * B_SHARD] = res.results[c]["out_t"].T
    return out


# revision 7
# speedup vs baseline: 1.5159x; 1.0598x over previous
"""Trainium2 Bass kernel for the Clifford EP model.

The reference model is entirely linear in x_mv:
  * Wx = geometric_product(x, W_in) is linear (Cayley-table contraction).
  * The free-phase relaxation h <- h + dt*(Wx - h), h0 = 0, has the exact
    closed form h_free = (1 - (1-dt)^N) * Wx.
  * The output is the scalar blade of geometric_product(h_free, W_out),
    and C[a, c, 0] != 0 only for c == a.

So the whole network collapses to a single matmul
    out[b, o] = X[b, :] @ Mf[:, o]
with X = x_mv.reshape(B, M*I) and a (M*I, O) folded weight matrix Mf that
only depends on W_in, W_out and the Cayley table.  The fold itself is tiny
(512x4096 @ 4096x64) and is done once on the host in float64; the device
does the batch-sized work: a data-parallel (1024x512)@(512x64) matmul per
NeuronCore, which is purely input-bandwidth bound.

Device layout: each core receives X_shard transposed (k on partitions) so
the TensorEngine can contract over k directly:
    psum[o, b] += Mf_chunk[k,o].T @ XT_chunk[k, b]
accumulated over 4 k-chunks of 128, with the 1024-batch free dim split in
two 512-wide matmuls (one PSUM bank each).

The device data path is fp16 (PSUM accumulation stays fp32): one PE pass
per matmul (fp32 needs LOW/HIGH double passes at half stream rate) and
half the DMA bytes.  Measured end-to-end relative error ~3e-4.
Set dtype="f32" in kernel() for the exact fp32 path.

Raw Bass (no TileContext) with manual semaphores: the Tile scheduler's
drain + double all-engine barrier + semaphore-clear tail costs ~7us,
which is material at this kernel size.
"""

import numpy as np

# Model constants (hardcoded per the problem spec).
B, M_DIM, I_B = 8192, 64, 8
H_DIM, O_DIM = 512, 64
K_DIM = M_DIM * I_B  # 512 contraction size
N_CORES = 8
B_SHARD = B // N_CORES  # 1024
KC = K_DIM // 128  # 4 contraction chunks
BH = B_SHARD // 512  # 2 moving-operand halves
DT, N_FREE = 0.1, 20
G_SIG = [1, 1, 1]

_CACHE = {}


def _cayley():
    n = len(G_SIG)
    I = 2**n
    C = np.zeros((I, I, I), dtype=np.float64)
    for a in range(I):
        for b in range(I):
            s = 0
            for i in range(n):
                if (b >> i) & 1:
                    s += bin(a >> (i + 1)).count("1")
            sign = (-1.0) ** s
            common = a & b
            for i in range(n):
                if (common >> i) & 1:
                    sign *= G_SIG[i]
            C[a, b, a ^ b] = sign
    return C


def _fold_weights(W_in, W_out):
    """Collapse W_in, W_out, Cayley table and the relaxation scale into
    a single (K_DIM, O_DIM) float64 matrix Mf with out = X @ Mf."""
    C = _cayley()
    I = I_B
    s = np.array([C[a, a, 0] for a in range(I)])  # scalar-blade signs
    coef = np.zeros((I, I))
    idx = np.zeros((I, I), dtype=np.int64)
    for a in range(I):
        for k in range(I):
            coef[a, k] = C[a, a ^ k, k]
            idx[a, k] = a ^ k
    W_in64 = np.asarray(W_in, dtype=np.float64)
    W_out64 = np.asarray(W_out, dtype=np.float64)
    # U[h, m, a, k] = C[a, a^k, k] * W_in[h, m, a^k]
    U = coef[None, None, :, :] * W_in64[:, :, idx]
    # W2[h, k, o] = s_k * W_out[o, h, k]
    W2 = s[None, :, None] * np.transpose(W_out64, (1, 2, 0))
    Uf = np.transpose(U, (1, 2, 0, 3)).reshape(M_DIM * I, H_DIM * I)
    c0 = 1.0 - (1.0 - DT) ** N_FREE
    return c0 * (Uf @ W2.reshape(H_DIM * I, O_DIM))


def _install_ntff_hook_shim():
    """This image's `antenv` lacks `axon_hooks`, which bass_utils imports
    when trace=True under axon.  Recreate it, wired to the ctypes NTFF
    profiler that trn_agent_boot ships.  No-op when the real module exists."""
    import sys
    import types

    try:
        import antenv.axon_hooks  # noqa: F401

        return
    except ImportError:
        pass
    try:
        import antenv
        from trn_agent_boot.trn_boot import _ntff_profile_via_ctypes

        hook = _ntff_profile_via_ctypes("/opt/axon/libaxon_pjrt.so")
    except Exception:
        antenv, hook = None, None
    if antenv is None:
        return
    mod = types.ModuleType("antenv.axon_hooks")
    mod.get_axon_ntff_profile_hook = lambda: hook
    mod.set_axon_ntff_profile_hook = lambda h: None
    sys.modules["antenv.axon_hooks"] = mod
    antenv.axon_hooks = mod


def _build_bass(dtype_key, n_warm):
    """Build the single-core SPMD program with raw-bass manual sync."""
    key = ("nc", dtype_key, n_warm)
    if key in _CACHE:
        return _CACHE[key]

    import concourse.bass as bass
    import concourse.mybir as mybir

    f32 = mybir.dt.float32
    dt_in = {"f16": mybir.dt.float16, "f32": f32, "bf16": mybir.dt.bfloat16}[
        dtype_key
    ]

    nc = bass.Bass("TRN2", debug=False)
    xt = nc.dram_tensor("xt", [K_DIM, B_SHARD], dt_in, kind="ExternalInput")
    # mf is host-prearranged to [128, KC*O_DIM] (contiguous 512B rows) —
    # loading the natural [512, 64] layout needs a 128B-element gather that
    # measured ~3.7us and gated the first matmul.
    mf = nc.dram_tensor("mf", [128, KC * O_DIM], dt_in, kind="ExternalInput")
    # Output is [2*64, 512]: batch-half stacked on partitions, so the copy
    # and store run at full 128-partition width.
    out_t = nc.dram_tensor("out_t", [BH * O_DIM, 512], f32, kind="ExternalOutput")

    with (
        nc.sbuf_tensor([128, KC, B_SHARD], dt_in) as xts,
        nc.sbuf_tensor([128, KC, O_DIM], dt_in) as mft,
        nc.sbuf_tensor([128, 512], mybir.dt.bfloat16) as warm_w,
        nc.sbuf_tensor([BH * O_DIM, 512], f32) as o_sb,
        nc.psum_tensor([BH * O_DIM, 512], f32) as ps,
        nc.psum_tensor([128, 512], f32) as warm_ps,
        nc.semaphore("sem_mf") as sem_mf,
        nc.semaphore("sem_warm") as sem_warm,
        nc.semaphore("sem_xt0") as sem_xt0,
        nc.semaphore("sem_xt1") as sem_xt1,
        nc.semaphore("sem_xt2") as sem_xt2,
        nc.semaphore("sem_xt3") as sem_xt3,
        nc.semaphore("sem_mm") as sem_mm,
        nc.semaphore("sem_cp") as sem_cp,
        nc.semaphore("sem_out") as sem_out,
        nc.Block() as block,
    ):
        sem_xt = [sem_xt0, sem_xt1, sem_xt2, sem_xt3]

        @block.gpsimd
        def _(gpsimd):
            gpsimd.memset(warm_w[:], 0.0).then_inc(sem_warm, 1)

        @block.sync
        def _(sync):
            # xt chunk loads; chunk order matches PE consumption order.
            for kc in (0, 1):
                sync.dma_start(
                    out=xts[:, kc, :], in_=xt[kc * 128 : (kc + 1) * 128, :]
                ).then_inc(sem_xt[kc], 16)
            sync.wait_ge(sem_cp, 1)
            sync.dma_start(out=out_t[:], in_=o_sb[:]).then_inc(sem_out, 16)
            sync.wait_ge(sem_out, 16)

        @block.scalar
        def _(scalar):
            # Second HWDGE issuer, in parallel with sync.
            scalar.dma_start(out=mft[:], in_=mf[:]).then_inc(sem_mf, 16)
            for kc in (2, 3):
                scalar.dma_start(
                    out=xts[:, kc, :], in_=xt[kc * 128 : (kc + 1) * 128, :]
                ).then_inc(sem_xt[kc], 16)

        @block.tensor
        def _(tensor):
            # Warm the PE HAM clock-gate while the DMAs stream.
            if n_warm:
                tensor.wait_ge(sem_warm, 1)
                for _ in range(n_warm):
                    nc.tensor.matmul(
                        warm_ps[:], warm_w[:, :128], warm_w[:], start=True, stop=True
                    )
            tensor.wait_ge(sem_mf, 16)
            for kc in range(KC):
                tensor.wait_ge(sem_xt[kc], 16)
                # The two batch halves go to separate PE column groups
                # (stationary cols 0-63 / 64-127) and run concurrently,
                # accumulating into one [128, 512] PSUM bank.
                for bh in range(BH):
                    mm = nc.tensor.matmul(
                        ps[bh * O_DIM : (bh + 1) * O_DIM, :],
                        mft[:, kc, :],
                        xts[:, kc, bh * 512 : (bh + 1) * 512],
                        start=(kc == 0),
                        stop=(kc == KC - 1),
                        tile_position=(0, bh * O_DIM),
                    )
                    if kc == KC - 1 and bh == BH - 1:
                        mm.then_inc(sem_mm, 1)

        @block.vector
        def _(vector):
            vector.wait_ge(sem_mm, 1)
            nc.vector.tensor_copy(o_sb[:], ps[:]).then_inc(sem_cp, 1)

    _CACHE[key] = nc
    return nc


def kernel(x_mv, W_in, W_out, trace=False, dtype="f16", n_warm=4, **trace_kwargs):
    _install_ntff_hook_shim()
    from concourse.bass_utils import run_bass_kernel_spmd

    np_dt = {"f16": np.float16, "f32": np.float32, "bf16": None}[dtype]
    if np_dt is None:
        import ml_dtypes

        np_dt = ml_dtypes.bfloat16

    x_mv = np.asarray(x_mv, dtype=np.float32)
    Mf = _fold_weights(W_in, W_out)
    # Device layout: mf[p, kc*O+o] = Mf[kc*128+p, o] (contiguous 512B rows).
    mf_dev = np.ascontiguousarray(
        Mf.reshape(KC, 128, O_DIM).transpose(1, 0, 2).reshape(128, KC * O_DIM),
        dtype=np_dt,
    )

    X = x_mv.reshape(B, K_DIM)
    in_maps = []
    for c in range(N_CORES):
        xs = np.ascontiguousarray(X[c * B_SHARD : (c + 1) * B_SHARD].T.astype(np_dt))
        in_maps.append({"xt": xs, "mf": mf_dev})

    nc = _build_bass(dtype, n_warm)
    res = run_bass_kernel_spmd(
        nc, in_maps, core_ids=list(range(N_CORES)), trace=trace, **trace_kwargs
    )
    _CACHE["last_results"] = res

    out = np.empty((B, O_DIM), dtype=np.float32)
    for c in range(N_CORES):
        # out_t is [BH*O, 512]: row bh*O+o, col j  ->  out[c*B_SHARD + bh*512 + j, o]
        ot = res.results[c]["out_t"].reshape(BH, O_DIM, 512)
        for bh in range(BH):
            out[c * B_SHARD + bh * 512 : c * B_SHARD + (bh + 1) * 512] = ot[bh].T
    return out
